# revision 20
# baseline (speedup 1.0000x reference)
"""Sparse attention TRN2 Bass kernel.

Problem: query/key/value (4, 16, 4096, 64) f32, sparsity_mask (16, 512) i32.
out[b,h,s,:] = softmax(Q[b,h,s] @ K_sparse[b,h].T / 8) @ V_sparse[b,h]
where K_sparse/V_sparse gather the 512 masked rows per head.

Distribution: the 64 (b, h) pairs are split 8-per-core across 8 NeuronCores
(batch+head data parallel, mask replicated per shard, no collectives).

Per-core device kernel, for each pair and each 512-query block:
  mm1 (f32r):  scoresT[k, q] = K_sT.T @ Q_T        (4 matmuls, N=512)
  exp (ACT):   expT = exp(scoresT / 8)             (PSUM -> SBUF, scale fused)
  mm2 (f32r):  outT_aug[65, 512] = V_aug.T @ expT  (accumulate over 4 k-chunks;
               V_aug has a ones column, so row 64 = sum_k exp = softmax denom)
  PE transpose outT_aug back to [q, d] orientation, then VectorE reciprocal +
  per-partition scale into a per-pair output tile, one 1 MiB DMA out per pair.
"""

import numpy as np

B, H, S, D, K = 4, 16, 4096, 64, 512
N_CORES = 8
PAIRS = B * H              # 64
PPC = PAIRS // N_CORES     # 8 pairs per core
QB = 512                   # queries per block
NBLK = S // QB             # 8 blocks per pair
NKC = K // 128             # 4 key chunks of 128
DA = D + 1                 # V augmented with ones column

_STATE = {}


def _build(repeat=1):
    import concourse.bacc as bacc
    import concourse.tile as tile
    import concourse.mybir as mybir
    from concourse.masks import make_identity

    f32 = mybir.dt.float32
    f32r = mybir.dt.float32r
    EXP = mybir.ActivationFunctionType.Exp

    nc = bacc.Bacc("TRN2", target_bir_lowering=False, debug=False,
                   num_devices=N_CORES)
    qT = nc.dram_tensor("qT", [PPC, D, S], f32r, kind="ExternalInput").ap()
    kT = nc.dram_tensor("kT", [PPC, D, K], f32r, kind="ExternalInput").ap()
    vaug = nc.dram_tensor("vaug", [PPC, K, DA], f32r, kind="ExternalInput").ap()
    out = nc.dram_tensor("out", [PPC, S, D], f32, kind="ExternalOutput").ap()

    with tile.TileContext(nc) as tc:
        from contextlib import ExitStack
        with ExitStack() as ctx:
            consts = ctx.enter_context(tc.tile_pool(name="consts", bufs=1))
            qpool = ctx.enter_context(tc.tile_pool(name="qpool", bufs=2))
            kpool = ctx.enter_context(tc.tile_pool(name="kpool", bufs=2))
            vpool = ctx.enter_context(tc.tile_pool(name="vpool", bufs=2))
            epool = ctx.enter_context(tc.tile_pool(name="epool", bufs=5))
            opool = ctx.enter_context(tc.tile_pool(name="opool", bufs=3))
            rpool = ctx.enter_context(tc.tile_pool(name="rpool", bufs=2))
            wpool = ctx.enter_context(tc.tile_pool(name="wpool", bufs=2))
            pscore = ctx.enter_context(
                tc.tile_pool(name="pscore", bufs=3, space="PSUM"))
            pmisc = ctx.enter_context(
                tc.tile_pool(name="pmisc", bufs=2, space="PSUM"))

            ident = consts.tile([DA, DA], f32)
            make_identity(nc, ident[:])

            def tail1(st):
                # block-(n-1): mm2 accumulate + PSUM->SBUF copy
                esb, v_t, out_t, blk, p = st
                pt = pmisc.tile([DA, QB], f32, tag="ptb", name="pt")
                for c in range(NKC):
                    s, j = divmod(c, 2)
                    nc.tensor.matmul(
                        pt[:],
                        v_t[:, c * DA:(c + 1) * DA],
                        esb[s][:, j * QB:(j + 1) * QB],
                        start=(c == 0), stop=(c == NKC - 1))

                osb = opool.tile([DA, QB], f32)
                nc.vector.tensor_copy(osb[:], pt[:])
                return (osb, out_t, blk, p)

            def tail2(st):
                # block-(n-2): transpose back to [q, d], normalize, store
                osb, out_t, blk, p = st
                trt = pmisc.tile([128, 4 * DA], f32, tag="ptb", name="trt")
                for j in range(4):
                    nc.tensor.matmul(
                        trt[:, j * DA:(j + 1) * DA],
                        osb[:, j * 128:(j + 1) * 128],
                        ident[:],
                        is_transpose=True,
                        start=(j == 0), stop=(j == 3))

                rec = rpool.tile([128, 4], f32)
                nc.vector.reciprocal(
                    rec[:],
                    trt[:].rearrange("p (c n) -> p c n", n=DA)[:, :, D])
                col = blk * 4 * D
                nc.vector.tensor_mul(
                    out_t[:, col:col + 4 * D]
                    .rearrange("p (c n) -> p c n", n=D),
                    trt[:].rearrange("p (c n) -> p c n", n=DA)[:, :, 0:D],
                    rec[:, :, None].broadcast_to([128, 4, D]))
                last_pair = p == PPC - 1
                step = 1 if last_pair else 2
                if blk % step == step - 1:
                    w = 512 * step // 2
                    ch = blk // step
                    nc.gpsimd.dma_start(
                        out[p, 2 * w * ch:2 * w * (ch + 1), :]
                        .rearrange("(c q) n -> q c n", q=128),
                        out_t[:, w * ch:w * (ch + 1)]
                        .rearrange("p (c n) -> p c n", n=D))

            pend1 = None
            pend2 = None
            for p_rep in range(repeat * PPC):
                p = p_rep % PPC
                # per-pair loads: K_sT, V_aug, and all of Q_T in one DMA each
                # kt/qt duplicated into partitions 64-127 so mm1 chunk
                # pairs run concurrently in both PE row-group halves
                kt_t = kpool.tile([128, K], f32r)
                nc.gpsimd.dma_start(kt_t[0:D, :], kT[p])
                nc.gpsimd.dma_start(kt_t[D:2 * D, :], kT[p])
                qt_t = qpool.tile([128, S], f32r)
                v_t = vpool.tile([128, NKC * DA], f32r)
                for qq in range(4):
                    sl = slice(qq * S // 4, (qq + 1) * S // 4)
                    nc.sync.dma_start(qt_t[0:D, sl], qT[p, :, sl])
                    nc.sync.dma_start(qt_t[D:2 * D, sl], qT[p, :, sl])
                    if qq == 0:
                        nc.sync.dma_start(
                            v_t[:].rearrange("p (c n) -> p c n", n=DA),
                            vaug[p].rearrange("(c p) n -> p c n", p=128))

                # per-pair output accumulator, one 1 MiB store at the end
                out_t = wpool.tile([128, NBLK * 4 * D], f32)

                for blk in range(NBLK):
                    q0 = blk * QB

                    # mm1: scoresT chunks [128k, 512q], two 2-bank psum tiles
                    psc = [pscore.tile([128, 2 * QB], f32, tag="psc",
                                       name="psc") for _ in range(2)]
                    for c in range(NKC):
                        s, j = divmod(c, 2)
                        rb = D * j
                        nc.tensor.matmul(
                            psc[s][:, j * QB:(j + 1) * QB],
                            kt_t[rb:rb + D, c * 128:(c + 1) * 128],
                            qt_t[rb:rb + D, q0:q0 + QB],
                            start=True, stop=True,
                            tile_position=(rb, 0))

                    # exp with fused 1/sqrt(D) scale
                    esb = [epool.tile([128, 2 * QB], f32r, tag="esb",
                                      name="esb") for _ in range(2)]
                    for s in range(2):
                        nc.scalar.activation(esb[s][:], psc[s][:], EXP,
                                             scale=0.125)

                    nxt = None
                    if pend1 is not None:
                        nxt = tail1(pend1)
                    if pend2 is not None:
                        tail2(pend2)
                    pend1 = (esb, v_t, out_t, blk, p)
                    pend2 = nxt

            nxt = tail1(pend1)
            if pend2 is not None:
                tail2(pend2)
            tail2(nxt)

    nc.compile()
    return nc


def _prep(query, key, value, sparsity_mask):
    q = np.ascontiguousarray(query, dtype=np.float32)
    k = np.ascontiguousarray(key, dtype=np.float32)
    v = np.ascontiguousarray(value, dtype=np.float32)
    m = np.asarray(sparsity_mask, dtype=np.int64)

    idx = m[None, :, :, None]
    kg = np.take_along_axis(k, idx, axis=2)        # (B, H, K, D)
    vg = np.take_along_axis(v, idx, axis=2)        # (B, H, K, D)

    qT = np.ascontiguousarray(
        q.reshape(PAIRS, S, D).transpose(0, 2, 1))  # (64, D, S)
    kTg = np.ascontiguousarray(
        kg.reshape(PAIRS, K, D).transpose(0, 2, 1))  # (64, D, K)
    va = np.empty((PAIRS, K, DA), dtype=np.float32)
    va[:, :, :D] = vg.reshape(PAIRS, K, D)
    va[:, :, D] = 1.0

    in_maps = []
    for i in range(N_CORES):
        sl = slice(i * PPC, (i + 1) * PPC)
        in_maps.append({
            "qT": np.ascontiguousarray(qT[sl]),
            "kT": np.ascontiguousarray(kTg[sl]),
            "vaug": np.ascontiguousarray(va[sl]),
        })
    return in_maps


def _run(in_maps, **kwargs):
    from concourse.bass_utils import run_bass_kernel_spmd
    if "nc" not in _STATE:
        _STATE["nc"] = _build()
    return run_bass_kernel_spmd(
        _STATE["nc"], in_maps, core_ids=list(range(N_CORES)), **kwargs)


def kernel(query, key, value, sparsity_mask):
    in_maps = _prep(query, key, value, sparsity_mask)
    res = _run(in_maps)
    full = np.concatenate([res.results[i]["out"] for i in range(N_CORES)],
                          axis=0)
    return full.reshape(B, H, S, D)

